# revision 27
# baseline (speedup 1.0000x reference)
"""Trainium2 Bass kernel for the 4-layer GCN diffusion denoiser (gnn_message_passing).

Strategy (8 NeuronCores, SPMD single program):
  - Nodes sharded 12500/core (padded to 12544 = 98*128). Edges routed to the core
    owning their dst node, grouped into 1024-dst windows x 4 source buckets.
  - Host precomputes x_t = (noise + temb + label) * dinv (time MLP + label
    embedding run on host), uploaded transposed per core.
  - Per layer, per-node features Hs = dinv * (X @ W) are stored as a bf16
    [100352, 128] table, AllGather'ed across cores (Shared-output collective).
  - Aggregation per (window, bucket): ONE bulk dma_gather (int16 indices into
    4 x 25088-row bucket views), segment-sum via PE matmuls against on-device
    one-hot matrices (iota == dst_local, plain 0/1 - dinv_dst is factored out
    and applied as a per-column multiply on the window aggregate).
  - Chunks of 128 edges may span 128-dst sub-tile boundaries; such chunks are
    matmul'ed once per sub-tile with a masked one-hot ("virtual chunks").
  - Self-loop term via eye-matmul against the previous layer's activation,
    which is kept resident in SBUF (no DMA reload).
  - Next layer's H production consumes the feature-major activation directly.

All cross-core communication is 4 AllGathers (one per layer boundary).
"""

import math
import sys
import types

import numpy as np

_N, _E, _D, _G = 100000, 1000000, 64, 128
_NCORES = 8
_SL = _N // _NCORES          # 12500 real nodes per core
_SLP = 12544                 # padded per-core slice (98*128)
_NP = _SLP * _NCORES         # 100352 padded table rows
_NBUCK = 4
_WIN = 1024
_NWIN = (_SLP + _WIN - 1) // _WIN    # 13 windows (last is 256 nodes)
# AllGather chunks: window-aligned quarters of the slice; bucket b of the
# gather = concat of all cores' quarter-b rows (<=32768, int16-addressable)
_QBOUNDS = [0, 4096, 7168, 10240]
_QSIZES = [4096, 3072, 3072, 2304]
_Q_OF_W = [0, 0, 0, 0, 1, 1, 1, 2, 2, 2, 3, 3, 3]
_NSUB = _SLP // 128          # 98 sub-tiles of 128 nodes
_F = 128
_PAD_DST = 1000.0
_MAX_CHUNKS_PER_CALL = 7     # <=896 idx per call: 16-engine split must stay
                             # within one 64-descriptor SWDGE packet

_compiled = {}


def _install_profile_shim():
    """Register the NTFF profile hook missing from this image's antenv."""
    try:
        import antenv
        from trn_agent_boot.trn_boot import _ntff_profile_via_ctypes
    except ImportError:
        return
    if "antenv.axon_hooks" in sys.modules:
        return
    mod = types.ModuleType("antenv.axon_hooks")
    hook = _ntff_profile_via_ctypes("/opt/axon/libaxon_pjrt.so")
    mod.get_axon_ntff_profile_hook = lambda: hook
    mod.set_axon_ntff_profile_hook = lambda h: None
    sys.modules["antenv.axon_hooks"] = mod
    antenv.axon_hooks = mod


def _silu_np(x):
    return x / (1.0 + np.exp(-x))


def _prep(inputs):
    """Host-side: x_t assembly, edge bucketing, gather indices, vc schedule."""
    src = np.asarray(inputs["edge_index"][0], dtype=np.int64)
    dst = np.asarray(inputs["edge_index"][1], dtype=np.int64)
    deg = np.bincount(dst, minlength=_N).astype(np.float32) + 1.0
    dinv = (1.0 / np.sqrt(deg)).astype(np.float32)

    # ---- x_t on host: noise + timestep embedding + label embedding, * dinv
    noise = np.asarray(inputs["noise_x"], np.float32)
    t_val = float(np.asarray(inputs["t"]).reshape(-1)[0])
    half = _D // 2
    freqs = np.exp(
        np.arange(half, dtype=np.float32) * (-math.log(10000.0) / (half - 1))
    ).astype(np.float32)
    args = np.float32(t_val) * freqs
    temb0 = np.concatenate([np.sin(args), np.cos(args)]).astype(np.float32)
    tw1 = np.asarray(inputs["time_w1"], np.float32)
    tw2 = np.asarray(inputs["time_w2"], np.float32)
    tb1 = np.asarray(inputs["time_b1"], np.float32)
    tb2 = np.asarray(inputs["time_b2"], np.float32)
    temb = _silu_np(temb0 @ tw1 + tb1) @ tw2 + tb2
    lab_emb = np.asarray(inputs["label_emb"], np.float32)
    x_t = noise + temb[None, :]
    lab = np.zeros((_N, _D), np.float32)
    lab[np.asarray(inputs["train_anm"])] = lab_emb[1]
    lab[np.asarray(inputs["train_norm"])] = lab_emb[0]
    x_t = (x_t + lab) * dinv[:, None]

    # ---- edge metadata
    core_of = dst // _SL
    dloc = dst % _SL
    w_of = dloc // _WIN
    st_of = (dloc % _WIN) // 128
    dstloc = (dloc % 128).astype(np.int32)
    dloc_src = src % _SL
    core_src = src // _SL
    qb = np.asarray(_QBOUNDS + [_SL], np.int64)
    b_of = np.searchsorted(qb, dloc_src, side="right") - 1
    qs = np.asarray(_QSIZES, np.int64)
    srow = core_src * qs[b_of] + (dloc_src - qb[b_of])
    i16 = srow.astype(np.int16)

    order = np.lexsort((srow, st_of, b_of, w_of, core_of))
    core_s = core_of[order]
    w_s = w_of[order]
    b_s = b_of[order]
    st_s = st_of[order]
    dstloc_s = dstloc[order]
    i16_s = i16[order]

    nwb = _NWIN * _NBUCK
    key = (core_s * _NWIN + w_s) * _NBUCK + b_s
    counts = np.bincount(key, minlength=_NCORES * nwb)
    run_start = np.zeros(_NCORES * nwb + 1, np.int64)
    np.cumsum(counts, out=run_start[1:])
    cnt_wb = counts.reshape(_NCORES, _NWIN, _NBUCK)

    ncw = np.ceil(cnt_wb / 128).astype(np.int64).max(axis=0)  # [NWIN, NBUCK]

    # vc union over cores: per (w,b) the set of (chunk, st) pairs
    vcs_wb = [[None] * _NBUCK for _ in range(_NWIN)]
    for w in range(_NWIN):
        for b in range(_NBUCK):
            pairs = set()
            for k in range(_NCORES):
                kk = (k * _NWIN + w) * _NBUCK + b
                s0, s1 = run_start[kk], run_start[kk + 1]
                if s1 == s0:
                    continue
                r = np.arange(s1 - s0)
                cs = np.unique(r // 128 * 16 + st_s[s0:s1])
                pairs.update(cs.tolist())
            vcs_wb[w][b] = [(int(p) // 16, int(p) % 16) for p in sorted(pairs)]

    # slot / vc base offsets (uniform across cores)
    slotbase = np.zeros((_NWIN, _NBUCK), np.int64)
    vcbase = np.zeros((_NWIN, _NBUCK), np.int64)
    sacc = vacc = 0
    for w in range(_NWIN):
        for b in range(_NBUCK):
            slotbase[w, b] = sacc
            vcbase[w, b] = vacc
            sacc += int(ncw[w, b]) * 128
            vacc += len(vcs_wb[w][b])
    tot_slots, nvc_tot = sacc, vacc

    # per-(w,b) map (chunk, st) -> vc column (relative)
    colmap = [[None] * _NBUCK for _ in range(_NWIN)]
    for w in range(_NWIN):
        for b in range(_NBUCK):
            m = np.full((max(int(ncw[w, b]), 1), 16), -1, np.int64)
            for j, (c, s) in enumerate(vcs_wb[w][b]):
                m[c, s] = j
            colmap[w][b] = m

    # ---- shared (weights) ----
    w_ = {m: np.asarray(inputs[m], np.float32) for m in
          ["w0", "b0", "w1", "b1", "w2", "b2", "w3", "b3"]}
    w1p = np.zeros((128, 128), np.float32); w1p[:, :64] = w_["w1"]
    w2p = np.zeros((128, 128), np.float32); w2p[:64, :] = w_["w2"]
    w3ap = np.zeros((128, 128), np.float32); w3ap[:, :64] = w_["w3"][:128]
    w3bp = np.zeros((128, 128), np.float32); w3bp[:, :64] = w_["w3"][128:]
    b1p = np.zeros((128, 1), np.float32); b1p[:64, 0] = w_["b1"]
    b3p = np.zeros((128, 1), np.float32); b3p[:64, 0] = w_["b3"]
    bf16 = np.float16  # device bf16 tensors are fed as np.float16 buffers

    shared = {
        "w0": w_["w0"].astype(bf16),
        "w1p": w1p.astype(bf16), "w2p": w2p.astype(bf16),
        "w3ap": w3ap.astype(bf16), "w3bp": w3bp.astype(bf16),
        "b0c": w_["b0"].reshape(128, 1).astype(np.float32),
        "b1c": b1p,
        "b2c": w_["b2"].reshape(128, 1).astype(np.float32),
        "b3c": b3p,
    }

    in_maps = []
    for k in range(_NCORES):
        idx_slots = np.zeros(tot_slots, np.int16)
        dstl = np.full((max(nvc_tot, 1), 128), _PAD_DST, np.float32)

        # place this core's edges
        for w in range(_NWIN):
            for b in range(_NBUCK):
                kk = (k * _NWIN + w) * _NBUCK + b
                s0, s1 = run_start[kk], run_start[kk + 1]
                n = s1 - s0
                if n == 0:
                    continue
                r = np.arange(n)
                slots = slotbase[w, b] + r
                idx_slots[slots] = i16_s[s0:s1]
                cols = vcbase[w, b] + colmap[w][b][r // 128, st_s[s0:s1]]
                dstl[cols, r % 128] = dstloc_s[s0:s1]

        wrapped = np.tile(idx_slots.reshape(-1, 16).T, (8, 1))

        nodes = np.arange(_SLP) + k * _SL
        nodes_c = np.minimum(nodes, _N - 1)
        sd = dinv[nodes_c].copy()
        sd[np.arange(_SLP) >= _SL] = 1.0
        selfdinv = sd.reshape(_NSUB, 128).T.copy()          # [128, 98] f32
        dinvb = np.broadcast_to(sd[None, :], (128, _SLP)).astype(bf16).copy()

        xt = np.zeros((_SLP, _D), np.float32)
        xt[:_SL] = x_t[k * _SL:(k + 1) * _SL]
        xtT = xt.T.astype(bf16).copy()                       # [64, 12544] bf16

        m = dict(shared)
        m.update({
            "midx": wrapped,
            "mdstl": dstl.T.astype(bf16).copy(),             # [128, nvc_tot]
            "mself": selfdinv,
            "mdinvb": dinvb,                                 # [128, 12544]
            "xtT": xtT,
        })
        in_maps.append(m)

    meta = (ncw, vcs_wb, slotbase, vcbase, tot_slots, nvc_tot)
    return in_maps, meta


def _build(meta):
    import concourse.bass as bass
    import concourse.bacc as bacc
    import concourse.tile as tile
    from concourse import mybir
    from concourse.masks import make_identity

    ncw, vcs_wb, slotbase, vcbase, tot_slots, nvc_tot = meta

    f32 = mybir.dt.float32
    bf16 = mybir.dt.float16
    AT = mybir.ActivationFunctionType
    OP = mybir.AluOpType

    nc = bacc.Bacc("TRN2", target_bir_lowering=False, debug=False,
                   num_devices=_NCORES, dynamic_dma_scratch_size=32768,
                   num_swdge_queues=4)

    din = {}
    def dt_in(name, shape, dt):
        din[name] = nc.dram_tensor(name, list(shape), dt, kind="ExternalInput")
        return din[name]

    dt_in("xtT", (_D, _SLP), bf16)
    dt_in("midx", (128, tot_slots // 16), mybir.dt.int16)
    dt_in("mdstl", (128, max(nvc_tot, 1)), bf16)
    dt_in("mself", (128, _NSUB), f32)
    dt_in("mdinvb", (128, _SLP), bf16)
    dt_in("w0", (64, 128), bf16)
    for nm in ["w1p", "w2p", "w3ap", "w3bp"]:
        dt_in(nm, (128, 128), bf16)
    for nm in ["b0c", "b1c", "b2c", "b3c"]:
        dt_in(nm, (128, 1), f32)
    out_d = nc.dram_tensor("out", [_SLP, 64], f32, kind="ExternalOutput")

    bnames = ["b0c", "b1c", "b2c", "b3c"]

    def _interleave(dram_tile, row0, nst, F):
        ap = dram_tile[:]
        return bass.AP(ap.tensor, ap.offset + row0 * F,
                       [[F, 128], [128 * F, nst], [1, F]])

    # Per-window matmul schedule (uniform across cores):
    #   [("seg", b, c, vc_rel, st), ...] in emission order, then selfs.
    scheds = []
    for w in range(_NWIN):
        ws = min(_WIN, _SLP - w * _WIN)
        nst = ws // 128
        items = []
        for b in range(_NBUCK):
            for j, (c, s) in enumerate(vcs_wb[w][b]):
                items.append(("seg", b, c, j, s))
        for s in range(nst):
            items.append(("self", 0, 0, 0, s))
        # start/stop flags per PSUM bank (bank = st//4 within the agg tile)
        banks = [it[4] // 4 for it in items]
        first, last = {}, {}
        for i, bk in enumerate(banks):
            first.setdefault(bk, i)
            last[bk] = i
        flags = [(i == first[bk], i == last[bk]) for i, bk in enumerate(banks)]
        scheds.append((ws, nst, items, flags))

    with tile.TileContext(nc) as tc:
        with tc.tile_pool(name="consts", bufs=1) as cp, \
             tc.tile_pool(name="meta", bufs=1) as mp, \
             tc.tile_pool(name="dram", bufs=1, space="DRAM") as dram, \
             tc.tile_pool(name="slices", bufs=2) as slicep, \
             tc.tile_pool(name="g", bufs=4) as gp, \
             tc.tile_pool(name="oh", bufs=2) as ohp, \
             tc.tile_pool(name="small", bufs=3) as sp, \
             tc.tile_pool(name="psA", bufs=2, space="PSUM") as psA, \
             tc.tile_pool(name="psB", bufs=2, space="PSUM") as psB:

            # ---- constants / metadata into SBUF ----
            def load(name, shape, dt, pool=cp):
                t = pool.tile(list(shape), dt, tag=name, name=name)
                nc.sync.dma_start(out=t[:], in_=din[name].ap())
                return t

            idx_t = load("midx", (128, tot_slots // 16), mybir.dt.int16, mp)
            dstl_t = load("mdstl", (128, max(nvc_tot, 1)), bf16, mp)
            self_t = load("mself", (128, _NSUB), f32)
            dinvb_t = load("mdinvb", (128, _SLP), bf16, mp)
            w0_t = load("w0", (64, 128), bf16)
            wl_t = {nm: load(nm, (128, 128), bf16)
                    for nm in ["w1p", "w2p", "w3ap", "w3bp"]}
            b_t = {nm: load(nm, (128, 1), f32) for nm in bnames}

            iota_i = cp.tile([128, 128], mybir.dt.int32, tag="iotai", name="iotai")
            nc.gpsimd.iota(iota_i[:], pattern=[[1, 128]], base=0, channel_multiplier=0)
            iota_b = cp.tile([128, 128], bf16, tag="iotab", name="iotab")
            nc.vector.tensor_copy(iota_b[:], iota_i[:])
            eye_t = cp.tile([128, 128], f32, tag="eye", name="eye")
            make_identity(nc, eye_t[:])
            eye_h = cp.tile([128, 128], bf16, tag="eyeh", name="eyeh")
            nc.vector.tensor_copy(eye_h[:], eye_t[:])

            # ---- DRAM working buffers (chunked by AllGather quarter) ----
            tableq_d = [[dram.tile([_QSIZES[q], _F], bf16, tag=f"tb{l}_{q}",
                                   name=f"tb{l}_{q}") for q in range(4)]
                        for l in range(4)]
            fullq_d = [[dram.tile([_NCORES * _QSIZES[q], _F], bf16, tag=f"fl{l}_{q}",
                                  name=f"fl{l}_{q}", addr_space="Shared")
                        for q in range(4)]
                       for l in range(4)]
            h0T_d = dram.tile([128, _SLP], bf16, tag="h0T", name="h0T")

            def write_table(l, w, nst, buf):
                q = _Q_OF_W[w]
                nc.sync.dma_start(
                    out=_interleave(tableq_d[l][q], w * _WIN - _QBOUNDS[q], nst, _F),
                    in_=buf)

            # ---- x0 phase: T0 = x_t @ w0 (dinv pre-folded on host) ----
            sbuf_prev = slicep.tile([128, _NSUB, _F], bf16, tag="slice", name="slice0")
            for w in range(_NWIN):
                ws, nst, _, _ = scheds[w]
                xt_in = sp.tile([64, ws], bf16, tag="xtin", name="xtin")
                nc.sync.dma_start(out=xt_in[:],
                                  in_=din["xtT"].ap()[:, w * _WIN:w * _WIN + ws])
                hps = psB.tile([128, ws], f32, tag="mm", name="hps0")
                for st in range(nst):
                    nc.tensor.matmul(hps[:, st * 128:(st + 1) * 128],
                                     lhsT=xt_in[:, st * 128:(st + 1) * 128],
                                     rhs=w0_t[:], start=True, stop=True,
                                     skip_group_check=True)
                for st in range(nst):
                    nc.scalar.activation(sbuf_prev[:, w * 8 + st, :],
                                         hps[:, st * 128:(st + 1) * 128], AT.Copy)
                write_table(0, w, nst, sbuf_prev[:, w * 8:w * 8 + nst, :])

            # ---- layers ----
            for layer in range(4):
                for q in range(4):
                    nc.gpsimd.collective_compute(
                        "AllGather", mybir.AluOpType.bypass,
                        replica_groups=[list(range(_NCORES))],
                        ins=[tableq_d[layer][q].opt()],
                        outs=[fullq_d[layer][q].opt()],
                    )
                if layer < 3:
                    sbuf_cur = slicep.tile([128, _NSUB, _F], bf16, tag="slice",
                                           name=f"slice{layer + 1}")
                # gather emission order: defer (w0,b3)/(w1,b3) so Q7 keeps
                # desc-genning other buckets while the last AllGather chunk
                # (q3, produced by the previous layer's final windows) lands
                glist = ([(0, b) for b in range(3)] + [(1, b) for b in range(3)]
                         + [(0, 3), (1, 3)]
                         + [(w, b) for w in range(2, _NWIN) for b in range(_NBUCK)])
                gtiles = {}
                gptr = [0]

                def emit_gather(w, b):
                    nch = int(ncw[w, b])
                    if nch == 0:
                        return
                    g = gp.tile([128, nch, _F], bf16, tag=f"g{b}", name=f"g{b}")
                    o16 = int(slotbase[w, b]) // 16
                    done = 0
                    while done < nch:
                        cc = min(_MAX_CHUNKS_PER_CALL, nch - done)
                        ni = cc * 128
                        nc.gpsimd.dma_gather(
                            out_ap=g[:, done:done + cc, :],
                            in_ap=fullq_d[layer][b][:],
                            idxs_ap=idx_t[:, o16 + done * 8:
                                          o16 + done * 8 + ni // 16],
                            num_idxs=ni, num_idxs_reg=ni, elem_size=_F,
                            queue_num=b,
                        )
                        done += cc
                    gtiles[(w, b)] = g

                def pump(upto_w):
                    while gptr[0] < len(glist) and glist[gptr[0]][0] <= upto_w:
                        emit_gather(*glist[gptr[0]])
                        gptr[0] += 1

                for w in range(_NWIN):
                    ws, nst, items, flags = scheds[w]
                    pump(w + 1)
                    g_b, oh_b = [None] * _NBUCK, [None] * _NBUCK
                    for b in range(_NBUCK):
                        if int(ncw[w, b]) == 0:
                            continue
                        nvc = len(vcs_wb[w][b])
                        oh = ohp.tile([128, nvc, 128], bf16, tag="oh", name="oh")
                        vb = int(vcbase[w, b])
                        iota_rep = bass.AP(
                            iota_b[:].tensor, iota_b[:].offset,
                            [list(iota_b[:].ap[0]), [0, nvc], [1, 128]])
                        nc.vector.tensor_tensor(
                            out=oh[:], in0=iota_rep,
                            in1=dstl_t[:, vb:vb + nvc][:, :, None]
                                .to_broadcast([128, nvc, 128]),
                            op=OP.is_equal)
                        g_b[b], oh_b[b] = gtiles.pop((w, b)), oh

                    agg = psA.tile([128, ws], f32, tag="agg", name="agg")
                    for it, (fst, lst) in zip(items, flags):
                        kind, b, c, j, s = it
                        if kind == "seg":
                            nc.tensor.matmul(agg[:, s * 128:(s + 1) * 128],
                                             lhsT=g_b[b][:, c, :],
                                             rhs=oh_b[b][:, j, :],
                                             start=fst, stop=lst,
                                             skip_group_check=True)
                        else:
                            nc.tensor.matmul(agg[:, s * 128:(s + 1) * 128],
                                             lhsT=sbuf_prev[:, w * 8 + s, :],
                                             rhs=eye_h[:],
                                             start=fst, stop=lst,
                                             skip_group_check=True)

                    # column scale by dinv_dst, then Silu(+bias)
                    xt_s = sp.tile([128, ws], f32, tag="xts", name="xts")
                    nc.vector.tensor_tensor(
                        out=xt_s[:], in0=agg[:],
                        in1=dinvb_t[:, w * _WIN:w * _WIN + ws], op=OP.mult)
                    if layer < 3:
                        xT = sp.tile([128, ws], bf16, tag="xT", name="xT")
                        nc.scalar.activation(xT[:], xt_s[:], AT.Silu,
                                             bias=b_t[bnames[layer]][:, :1])
                        if layer == 0:
                            nc.sync.dma_start(
                                out=h0T_d[:, w * _WIN:w * _WIN + ws], in_=xT[:])
                        hps = psB.tile([128, ws], f32, tag="mm", name="hps")
                        if layer == 2:
                            h0b = sp.tile([128, ws], bf16, tag="h0b", name="h0b")
                            nc.sync.dma_start(
                                out=h0b[:],
                                in_=h0T_d[:, w * _WIN:w * _WIN + ws])
                        for st in range(nst):
                            if layer < 2:
                                nc.tensor.matmul(
                                    hps[:, st * 128:(st + 1) * 128],
                                    lhsT=xT[:, st * 128:(st + 1) * 128],
                                    rhs=wl_t["w1p" if layer == 0 else "w2p"][:],
                                    start=True, stop=True, skip_group_check=True)
                            else:
                                nc.tensor.matmul(
                                    hps[:, st * 128:(st + 1) * 128],
                                    lhsT=xT[:, st * 128:(st + 1) * 128],
                                    rhs=wl_t["w3ap"][:],
                                    start=True, stop=False, skip_group_check=True)
                                nc.tensor.matmul(
                                    hps[:, st * 128:(st + 1) * 128],
                                    lhsT=h0b[:, st * 128:(st + 1) * 128],
                                    rhs=wl_t["w3bp"][:],
                                    start=False, stop=True, skip_group_check=True)
                        for st in range(nst):
                            nc.scalar.activation(
                                sbuf_cur[:, w * 8 + st, :],
                                hps[:, st * 128:(st + 1) * 128], AT.Copy,
                                scale=self_t[:, w * 8 + st:w * 8 + st + 1])
                        write_table(layer + 1, w, nst,
                                    sbuf_cur[:, w * 8:w * 8 + nst, :])
                    else:
                        xT3 = sp.tile([128, ws], f32, tag="xT3", name="xT3")
                        nc.scalar.activation(xT3[:], xt_s[:], AT.Silu,
                                             bias=b_t["b3c"][:, :1])
                        for st in range(nst):
                            tr = psB.tile([128, 128], f32, tag="mm", name="tr")
                            nc.tensor.transpose(tr[:],
                                                in_=xT3[:, st * 128:(st + 1) * 128],
                                                identity=eye_t[:])
                            oc = sp.tile([128, 64], f32, tag="oc", name="oc")
                            nc.vector.tensor_copy(oc[:], tr[:, 0:64])
                            row0 = w * _WIN + st * 128
                            nc.sync.dma_start(
                                out=out_d.ap()[row0:row0 + 128, :], in_=oc[:])
                if layer < 3:
                    sbuf_prev = sbuf_cur

    nc.compile()
    return nc


def _get_compiled(inputs):
    in_maps, meta = _prep(inputs)
    ncw, vcs_wb = meta[0], meta[1]
    key = (ncw.tobytes(), repr(vcs_wb))
    if key not in _compiled:
        _compiled[key] = _build(meta)
    return _compiled[key], in_maps


def _run(inputs, trace=False):
    _install_profile_shim()
    from concourse import bass_utils
    nc, in_maps = _get_compiled(inputs)
    res = bass_utils.run_bass_kernel_spmd(
        nc, in_maps, core_ids=list(range(_NCORES)), trace=trace)
    out = np.concatenate([res.results[k]["out"][:_SL] for k in range(_NCORES)], axis=0)
    return out[:_N].astype(np.float32), res.exec_time_ns


def kernel(**inputs):
    out, _ = _run(inputs, trace=False)
    return out
